# revision 2
# baseline (speedup 1.0000x reference)
"""Trainium2 Bass kernel for strided Conv2d + stride-permutation + bias.

Problem (hardcoded):
  x      [16, 256, 64, 64] f32
  weight [256, 256, 3, 3]  f32  (OIHW)
  bias   [256]             f32
  conv: stride (2,2), padding (1,1), dilation (1,1) -> [16, 256, 32, 32]
  output: spatial flattened and permuted into the 4 stride-phase groups
          (si, sj, i, j) order, + bias -> [16, 256, 1024]

Strategy: data-parallel over batch across 8 cores (2 images/core), with
one level of Strassen over the 2x2x2 block structure (cout-tiles x
cin-tiles x images): 7 block-multiplies instead of 8, i.e. 7/8 of the
PE columns of the direct kernel (126 vs 144 matmuls of 512 cols per
core per pass).

  A = weights as 2x2 blocks over (cot, cit)       [per 3x3 tap]
  B = x as 2x2 blocks over (cit, img)             [core's 2 images]
  C = out as 2x2 blocks over (cot, img)

All A/B-side Strassen combines are precomputed on the host (f16); each
M_i accumulates its 9 taps (phase-split stride-2 conv addressing, same
as the direct kernel) into its own PSUM bank.  The C-side combines run
on DVE with at most one PSUM operand per op (walrus restriction),
ordered so every PSUM bank frees right after its M completes - the PE
stream never waits.  The final combine fuses bias-add and scatters
into the stride-permuted output layout, making the store to HBM fully
contiguous.

Numerics: f16 inputs + one Strassen level measures rel_err ~6.9e-4 on
hardware (gate 2e-2).
"""

import os
import time

import numpy as np

_B, _C, _H, _W = 16, 256, 64, 64
_HO = _WO = 32
_NCORES = 8
_IMGS = _B // _NCORES  # images per core
_PL = 34  # padded phase-plane side

# tap index (0,1,2) -> (row/col phase, start offset in padded plane)
_TAP = {0: (1, 0), 1: (0, 1), 2: (1, 1)}

# taps ordered by phase plane (3, 2, 1, 0)
_TAP_ORDER = [
    (0, 0), (0, 2), (2, 0), (2, 2),  # phase (1,1) = plane 3
    (0, 1), (2, 1),                  # phase (1,0) = plane 2
    (1, 0), (1, 2),                  # phase (0,1) = plane 1
    (1, 1),                          # phase (0,0) = plane 0
]

_PROG_CACHE = {}


def _build_program(reps: int):
    import concourse.tile as tile
    from concourse import bacc, mybir

    f32 = mybir.dt.float32
    f16 = mybir.dt.float16
    ADD = mybir.AluOpType.add

    nc = bacc.Bacc("TRN2", target_bir_lowering=False, debug=False)

    # B-combos: [combo 7, cip 128, ph 4, 34, 34]
    xB = nc.dram_tensor(
        "xB", [7, 128, 4, _PL, _PL], f16, kind="ExternalInput"
    ).ap()
    # W-combos: [cip 128, combo 7, tap 9, cop 128]
    wt = nc.dram_tensor("wt", [128, 7, 9, 128], f16, kind="ExternalInput").ap()
    bs = nc.dram_tensor("bs", [128, 2], f32, kind="ExternalInput").ap()
    out = nc.dram_tensor("out", [_IMGS, 2, 128, 1024], f32, kind="ExternalOutput").ap()

    with tile.TileContext(nc) as tc:
        with (
            tc.tile_pool(name="const", bufs=1) as constp,
            tc.tile_pool(name="xbuf", bufs=1) as xp,
            tc.tile_pool(name="scr", bufs=2) as scrp,
            tc.tile_pool(name="obuf", bufs=2) as obp,
            tc.tile_pool(name="psum", bufs=8, space="PSUM") as psp,
        ):
            wtile = constp.tile([128, 7, 9, 128], f16)
            btile = constp.tile([128, 2], f32)
            xt = {}
            for m in range(7):
                xt[m] = xp.tile([128, 4, _PL, _PL], f16, tag=f"b_{m}", name=f"b_{m}")

            # alternate DMAs between the two HWDGE rings (SP + ACT)
            _eng = [nc.sync, nc.scalar]
            _ei = [0]

            def _dma(dst, src):
                _eng[_ei[0] & 1].dma_start(dst, src)
                _ei[0] += 1

            _dma(wtile[:], wt[:])
            _dma(btile[:], bs[:])
            for m in range(7):
                _dma(xt[m][:], xB[m])

            def evict(src1, bias, src2, ob, half):
                # ob slice <- src1 + bias + src2, scattered into the
                # stride-permuted output layout (si,sj,i,j)
                for si in range(2):
                    dst = ob[:, si, :, half * 8 : half * 8 + 8, :].rearrange(
                        "p sj rh j -> p rh j sj"
                    )
                    nc.vector.scalar_tensor_tensor(
                        dst,
                        src1[:, si : 16 : 2, :],
                        bias,
                        src2[:, si : 16 : 2, :],
                        ADD,
                        ADD,
                    )

            # Strassen recipe (0-indexed M):
            # C(img0,cot0)=M0+M3-M4+M6   C(img1,cot0)=M2+M4
            # C(img0,cot1)=M1+M3         C(img1,cot1)=M0-M1+M2+M5
            _M_ORDER = [0, 1, 3, 2, 4, 5, 6]

            for _rep in range(reps):
                ob = {}
                for img in range(_IMGS):
                    for cot in range(2):
                        ob[(img, cot)] = obp.tile(
                            [128, 2, 2, 16, 16], f32,
                            tag=f"ob{img}{cot}", name=f"ob{img}{cot}",
                        )
                for half in range(2):
                    ps = {}
                    t11 = scrp.tile([128, 16, 32], f32, tag="t11", name="t11")
                    t22 = scrp.tile([128, 16, 32], f32, tag="t22", name="t22")
                    s0 = scrp.tile([128, 16, 32], f32, tag="s0", name="s0")
                    s1 = scrp.tile([128, 16, 32], f32, tag="s1", name="s1")
                    s2 = scrp.tile([128, 16, 32], f32, tag="s2", name="s2")
                    for m in _M_ORDER:
                        ps[m] = psp.tile([128, 16, 32], f32, tag="ps", name="ps")
                        for ti, (kh, kw) in enumerate(_TAP_ORDER):
                            phr, r0 = _TAP[kh]
                            phc, c0 = _TAP[kw]
                            rhs = xt[m][
                                :,
                                phr * 2 + phc,
                                r0 + half * 16 : r0 + half * 16 + 16,
                                c0 : c0 + 32,
                            ]
                            nc.tensor.matmul(
                                ps[m][:],
                                wtile[:, m, kh * 3 + kw, :],
                                rhs,
                                start=(ti == 0),
                                stop=(ti == 8),
                            )
                        # incremental DVE combines; each op reads at most
                        # one PSUM operand, and each bank frees right
                        # after its M completes
                        if m == 0:
                            nc.vector.tensor_copy(s0[:], ps[0][:])
                        if m == 1:
                            nc.vector.tensor_copy(s1[:], ps[1][:])
                            nc.vector.tensor_sub(t22[:], s0[:], ps[1][:])
                        if m == 3:
                            nc.vector.tensor_add(t11[:], s0[:], ps[3][:])
                            evict(s1, btile[:, 1:2], ps[3], ob[(0, 1)], half)
                        if m == 2:
                            nc.vector.tensor_copy(s2[:], ps[2][:])
                            nc.vector.tensor_add(t22[:], t22[:], ps[2][:])
                        if m == 4:
                            nc.vector.tensor_sub(t11[:], t11[:], ps[4][:])
                            evict(s2, btile[:, 0:1], ps[4], ob[(1, 0)], half)
                        if m == 5:
                            evict(t22, btile[:, 1:2], ps[5], ob[(1, 1)], half)
                        if m == 6:
                            evict(t11, btile[:, 0:1], ps[6], ob[(0, 0)], half)
                for img in range(_IMGS):
                    for cot in range(2):
                        _eng[(img * 2 + cot) % 2].dma_start(
                            out[img, cot], ob[(img, cot)][:]
                        )

    nc.compile()
    return nc


def _get_program(reps: int):
    if reps not in _PROG_CACHE:
        _PROG_CACHE[reps] = _build_program(reps)
    return _PROG_CACHE[reps]


def _prep_inputs(x, weight, bias):
    x = np.ascontiguousarray(np.asarray(x, dtype=np.float32))
    weight = np.ascontiguousarray(np.asarray(weight, dtype=np.float32))
    bias = np.ascontiguousarray(np.asarray(bias, dtype=np.float32))

    xr = x.reshape(_B, 2, 128, _H, _W)  # [B, cit, cip, H, W]

    # per-core B-combos over (cit, img); B block rows = cit, cols = img
    # B0=(c0,i0)+(c1,i1) B1=(c0,i0) B2=(c0,i1)-(c1,i1) B3=(c1,i0)-(c0,i0)
    # B4=(c1,i1) B5=(c0,i0)+(c0,i1) B6=(c1,i0)+(c1,i1)
    def combos(x0, x1):  # per-image [2cit, 128, H, W]
        return np.stack([
            x0[0] + x1[1],
            x0[0],
            x1[0] - x1[1],
            x0[1] - x0[0],
            x1[1],
            x0[0] + x1[0],
            x0[1] + x1[1],
        ])  # [7, 128, H, W]

    in_maps = []
    for c in range(_NCORES):
        cb = combos(xr[2 * c], xr[2 * c + 1])
        xBc = np.zeros((7, 128, 4, _PL, _PL), dtype=np.float16)
        for rp in range(2):
            for cp in range(2):
                xBc[:, :, rp * 2 + cp, 1:33, 1:33] = cb[:, :, rp::2, cp::2]
        in_maps.append({"xB": np.ascontiguousarray(xBc)})

    # W-combos as lhsT [cip, combo, tap, cop]; A block rows = cot, cols = cit
    w6 = weight.reshape(2, 128, 2, 128, 3, 3)  # [cot, cop, cit, cip, kh, kw]
    wtr = w6.transpose(3, 2, 0, 4, 5, 1)  # [cip, cit, cot, kh, kw, cop]
    A = lambda r, c: wtr[:, c, r]  # [cip, kh, kw, cop]
    Wc = np.stack([
        A(0, 0) + A(1, 1),
        A(1, 0) + A(1, 1),
        A(0, 0),
        A(1, 1),
        A(0, 0) + A(0, 1),
        A(1, 0) - A(0, 0),
        A(0, 1) - A(1, 1),
    ], axis=1)  # [cip, 7, kh, kw, cop]
    wt = np.ascontiguousarray(Wc.reshape(128, 7, 9, 128).astype(np.float16))

    bs = np.ascontiguousarray(bias.reshape(2, 128).T)  # [cop, cot]

    for m in in_maps:
        m["wt"] = wt
        m["bs"] = bs
    return in_maps


class _Runner:
    """Persistent jitted SPMD executor for one built program (one `reps`
    value). Mirrors bass2jax.run_bass_via_pjrt but keeps the jitted
    callable so repeat calls skip retrace/recompile, and lets callers
    pre-place inputs on device for clean timing."""

    def __init__(self, nc):
        import jax
        import numpy as _np
        from jax.sharding import Mesh, NamedSharding, PartitionSpec
        from jax.experimental.shard_map import shard_map
        import concourse.mybir as mybir
        from concourse import bass2jax

        bass2jax.install_neuronx_cc_hook()
        self.jax = jax
        self.nc = nc

        partition_name = (
            nc.partition_id_tensor.name if nc.partition_id_tensor else None
        )
        in_names, out_names, out_avals, zero_outs = [], [], [], []
        for alloc in nc.m.functions[0].allocations:
            if not isinstance(alloc, mybir.MemoryLocationSet):
                continue
            name = alloc.memorylocations[0].name
            if alloc.kind == "ExternalInput":
                if name != partition_name:
                    in_names.append(name)
            elif alloc.kind == "ExternalOutput":
                shape = tuple(alloc.tensor_shape)
                dtype = mybir.dt.np(alloc.dtype)
                out_names.append(name)
                out_avals.append(jax.core.ShapedArray(shape, dtype))
                zero_outs.append(_np.zeros(shape, dtype))
        self.in_names = in_names
        self.out_names = out_names
        self.out_avals = out_avals
        self.zero_outs = zero_outs
        n_params = len(in_names)

        def _body(*args):
            operands = list(args)
            if partition_name is not None:
                operands.append(bass2jax.partition_id_tensor())
            outs = bass2jax._bass_exec_p.bind(
                *operands,
                out_avals=tuple(out_avals),
                in_names=tuple(in_names + out_names + ([partition_name] if partition_name else [])),
                out_names=tuple(out_names),
                lowering_input_output_aliases=(),
                sim_require_finite=True,
                sim_require_nnan=True,
                nc=nc,
            )
            return tuple(outs)

        devices = jax.devices()[:_NCORES]
        self.mesh = Mesh(np.asarray(devices), ("core",))
        self.spec = NamedSharding(self.mesh, PartitionSpec("core"))
        n_outs = len(out_names)
        in_specs = (PartitionSpec("core"),) * (n_params + n_outs)
        out_specs = (PartitionSpec("core"),) * n_outs
        self.fn = jax.jit(
            shard_map(
                _body,
                mesh=self.mesh,
                in_specs=in_specs,
                out_specs=out_specs,
                check_rep=False,
            ),
            keep_unused=True,
        )

    def place_inputs(self, in_maps):
        concat = [
            np.concatenate([np.asarray(m[name]) for m in in_maps], axis=0)
            for name in self.in_names
        ]
        return [self.jax.device_put(a, self.spec) for a in concat]

    def place_zeros(self):
        return [
            self.jax.device_put(
                np.zeros((_NCORES * z.shape[0], *z.shape[1:]), z.dtype), self.spec
            )
            for z in self.zero_outs
        ]

    def __call__(self, dev_inputs, dev_zeros):
        outs = self.fn(*dev_inputs, *dev_zeros)
        self.jax.block_until_ready(outs)
        return outs


_RUNNER_CACHE = {}


def _get_runner(reps: int) -> "_Runner":
    if reps not in _RUNNER_CACHE:
        _RUNNER_CACHE[reps] = _Runner(_get_program(reps))
    return _RUNNER_CACHE[reps]


def _run(in_maps, reps: int):
    r = _get_runner(reps)
    dev_in = r.place_inputs(in_maps)
    dev_z = r.place_zeros()
    t0 = time.perf_counter()
    outs = r(dev_in, dev_z)
    dt = time.perf_counter() - t0
    full = np.asarray(outs[0]).reshape(_NCORES, _IMGS, 2, 128, 1024)
    return full.reshape(_B, _C, 1024), dt


def kernel(x, weight, bias):
    in_maps = _prep_inputs(x, weight, bias)
    reps = int(os.environ.get("BASS_CONV_REPS", "1"))
    out, _ = _run(in_maps, reps)
    return out
